# revision 13
# baseline (speedup 1.0000x reference)
"""Multi-head attention (B=2, S=4096, D=512, H=8) on 8 trn2 NeuronCores.

v3 sharding: head-pair tensor parallel. Core c = (batch c//4, head-pair
c%4). Each core projects K^T/V/Q^T for only its 2 heads over the full
sequence (no replicated projection work), runs causal flash attention for
those heads over all 4096 queries, and applies its 128-row slice of Wo to
produce a PARTIAL output [S, D]. The host sums the 4 partials per batch
and adds bo (no on-device collectives).

Why this layout wins over query-sharding:
  - K/V/Q projection work per core drops 4x (was replicated across the
    4 cores of a batch).
  - Causal structure is exact per core (single triangular mask tile at
    the diagonal block) -- no cross-core band masks, no interleaving.
  - ACT-engine exp is the critical path (~150us/core): exp instructions
    cover TWO kc blocks per issue via a bank-aligned 3D PSUM AP
    [128, 2, 512] (measured: one 2-bank exp = 1113ns vs 2x687ns).
  - Q/K projections run as fp8e4 DoubleRow matmuls (2 k-tiles per
    instruction, 2x throughput; weights prescaled x16 so fp8 range is
    used, compensated in the exp scale 0.125/256). V/Wo stay bf16:
    numpy simulation shows fp8 V/Wo break the 2e-2 rel-err budget
    (early causal queries average too few keys to wash out fp8 noise).
  - QK matmuls for the two heads sit at PE partition offsets 0/64 and
    are issued adjacently so the row-tiled PE can overlap them.
  - Projections stream through a global work queue interleaved into the
    attention groups, paced by ranked stage labels (T/Q/K/V0-V3) so each
    QK/PV reads only data whose writes are already issued (tile-framework
    deps are program-order), while filling PE bubbles under the ACT exp
    shadow. Attention starts as soon as proj(0) reaches its K stage.

PSUM budget (8 banks): qk scores [128,2,512]f32 x2 heads = 4, pv
accumulators [65,512]f32 x2 = 2, misc rotating [128,512]f32 x2 = 2
(transpose/proj/out-proj/broadcast all share the misc pool).

Hard-won HW constraints honored (sim passes but HW fails if not):
  - ACT APs may span PSUM banks only with bank-aligned 3D APs (runs must
    not cross a bank boundary),
  - DoubleRow ldweights tile stride must be a multiple of 16 bytes,
  - GPSIMD (Pool) cannot touch PSUM; PSUM drains go through DVE,
  - DVE ops must not read and write the same SBUF range in-place,
  - two matmuls must not write disjoint ranges of one PSUM bank.
"""

import numpy as np

# Problem dims (hardcoded per contract)
B, S, D, H, PD = 2, 4096, 512, 8, 64
P = 128
NCORES = 8
CPB = 4            # cores per batch (= head-pairs)
QC = 512           # attention q-chunk width
NQC = S // QC      # 8 chunks
SC = 512           # projection sequence chunk
NSC = S // SC      # 8
NKT = S // P       # 32 key subblocks of 128
DC = D // P        # 4 d-chunks of 128
HP = H // 2        # 4 head-pairs
QR = S // CPB      # (generic path) query rows per core

_prog_cache = {}
_bo_stash = {}


def _build_tril_v3():
    import concourse.mybir as mybir
    import concourse.tile as tile
    from concourse import bacc
    from concourse.masks import make_identity

    f32 = mybir.dt.float32
    bf16 = mybir.dt.bfloat16
    fp8 = mybir.dt.float8e4
    Exp = mybir.ActivationFunctionType.Exp
    DR = mybir.MatmulPerfMode.DoubleRow
    Alu = mybir.AluOpType
    ESCALE = 0.125 / 256.0   # 1/sqrt(64), /16^2 for prescaled Wq/Wk

    nc = bacc.Bacc(debug=False, target_bir_lowering=False)

    xb_d = nc.declare_dram_parameter("xb", [S, D], f32, isOutput=False)
    wq_d = nc.declare_dram_parameter("wq", [P, DC, P], fp8, isOutput=False)
    wk_d = nc.declare_dram_parameter("wk", [P, DC, P], fp8, isOutput=False)
    wv_d = nc.declare_dram_parameter("wv", [P, DC, P], bf16, isOutput=False)
    wo_d = nc.declare_dram_parameter("wo", [P, D], bf16, isOutput=False)
    bq_d = nc.declare_dram_parameter("bq", [P, SC], f32, isOutput=False)
    bk_d = nc.declare_dram_parameter("bk", [P, SC], f32, isOutput=False)
    bv_d = nc.declare_dram_parameter("bv", [P, 2, PD], f32, isOutput=False)
    tri_d = nc.declare_dram_parameter("tri", [P, P], bf16, isOutput=False)
    onesc_d = nc.declare_dram_parameter("onesc", [1, PD], bf16, isOutput=False)
    out_d = nc.declare_dram_parameter("out", [S, D], f32, isOutput=True)

    with tile.TileContext(nc) as tc, nc.allow_low_precision(
            reason="bf16/fp8 matmul operands; fp32 PSUM accumulation"):
        with (
            tc.tile_pool(name="const", bufs=1) as constp,
            tc.tile_pool(name="big", bufs=1) as bigp,
            tc.tile_pool(name="work", bufs=4) as work,
            tc.tile_pool(name="p2", bufs=3) as p2,
            tc.tile_pool(name="p2s", bufs=2) as p2s,
            tc.tile_pool(name="qkps", bufs=1, space="PSUM") as qkps,
            tc.tile_pool(name="pvps", bufs=1, space="PSUM") as pvps,
            tc.tile_pool(name="bps", bufs=2, space="PSUM") as bps,
        ):
            ident = constp.tile([P, P], f32, tag="ident")
            make_identity(nc, ident)
            identb = constp.tile([P, P], bf16, tag="identb")
            nc.vector.tensor_copy(out=identb[:], in_=ident[:])
            tri = constp.tile([P, P], bf16, tag="tri")
            ones_col = constp.tile([1, PD], bf16, tag="onesc")
            wq = constp.tile([P, DC, P], fp8, tag="wq")
            wk = constp.tile([P, DC, P], fp8, tag="wk")
            wv = constp.tile([P, DC, P], bf16, tag="wv")
            wo = constp.tile([P, D], bf16, tag="wo")
            bq = constp.tile([P, SC], f32, tag="bq")
            bk = constp.tile([P, SC], f32, tag="bk")
            bv = constp.tile([P, 2, PD], f32, tag="bv")

            # K^T/Q^T: partitions 0-63 = head0 dims, 64-127 = head1 dims
            kts = bigp.tile([P, S], bf16, tag="kts")
            qt = bigp.tile([P, S], bf16, tag="qt")
            # V rows + ones column per (kc, head); bf16 for diagonal
            # blocks (exact), fp8 for far blocks (DoubleRow PV)
            vts = bigp.tile([P, NKT, 2, PD + 1], bf16, tag="vts")
            vts8 = bigp.tile([P, NKT, 2, 72], fp8, tag="vts8")

            def dma_in(sci):
                # gpsimd queue supports casting DMAs: f32 DRAM -> bf16 SBUF
                xraw = work.tile([P, SC // P, D], bf16, tag="xraw",
                                 name=f"xraw{sci}")
                nc.gpsimd.dma_start(
                    xraw[:],
                    xb_d[sci * SC:(sci + 1) * SC].rearrange(
                        "(rt p) d -> p rt d", p=P),
                )
                return xraw

            def proj_steps(sci, xraw, pool):
                """Generator: projection of X chunk sci; yields stage
                labels as pieces complete (T/Q/K/V)."""
                xt8 = work.tile([P, DC, SC], fp8, tag="xt8",
                                name=f"xt8{sci}")
                xtb = work.tile([P, DC, SC], bf16, tag="xtb",
                                name=f"xtb{sci}")
                # transpose pieces (bf16: 1 cycle/row on the PE)
                for half in range(2):
                    for rt in range(2 * half, 2 * half + 2):
                        pst = pool.tile([P, D], bf16, tag="b",
                                        name=f"pst{sci}_{rt}")
                        for dc in range(DC):
                            nc.tensor.transpose(
                                pst[:, dc * P:(dc + 1) * P],
                                xraw[:, rt, dc * P:(dc + 1) * P],
                                identb[:],
                            )
                        nc.vector.tensor_copy(
                            out=xtb[:, :, rt * P:(rt + 1) * P],
                            in_=pst[:].rearrange("p (dc j) -> p dc j", dc=DC),
                        )
                        # fp8 casts alternate Pool/DVE: Pool's ~1.9us
                        # per cast otherwise serializes 7.5us ahead of
                        # the K/Q DoubleRow matmuls in forced bundles
                        ceng = nc.gpsimd if rt % 2 == 0 else nc.vector
                        ceng.tensor_copy(
                            out=xt8[:, :, rt * P:(rt + 1) * P],
                            in_=xtb[:, :, rt * P:(rt + 1) * P],
                        )
                    yield 'T'
                # Q^T first (chunk j's attention needs qt(j) before kts(j)),
                # then K^T, via fp8 DoubleRow
                for lab, w8, bias, dst in (('Q', wq, bq, qt),
                                           ('K', wk, bk, kts)):
                    psx = pool.tile([P, D], f32, tag="b", name=f"ps{sci}")
                    for di in range(2):
                        nc.tensor.matmul(
                            psx[:, 0:SC],
                            w8[:, 2 * di:2 * di + 2, :],
                            xt8[:, 2 * di:2 * di + 2, :],
                            start=(di == 0), stop=(di == 1),
                            perf_mode=DR,
                        )
                    # drains stay off ACT: an in-queue IDENTITY blocks
                    # later exps on psx completion (head-of-line)
                    nc.vector.tensor_add(
                        out=dst[:, sci * SC:(sci + 1) * SC],
                        in0=psx[:, 0:SC], in1=bias[:])
                    yield lab
                # V (bf16, exact) per row-tile
                for rt in range(SC // P):
                    psv = pool.tile([P, P], f32, tag="b",
                                    name=f"psv{sci}_{rt}")
                    for dc in range(DC):
                        nc.tensor.matmul(
                            psv[:],
                            xtb[:, dc, rt * P:(rt + 1) * P],
                            wv[:, dc, :],
                            start=(dc == 0), stop=(dc == DC - 1),
                        )
                    kti = sci * (SC // P) + rt
                    nc.vector.tensor_add(
                        out=vts[:, kti, :, 0:PD],
                        in0=psv[:].rearrange("p (h d) -> p h d", h=2),
                        in1=bv[:],
                    )
                    nc.gpsimd.tensor_copy(
                        out=vts8[:, kti, :, 0:PD],
                        in_=vts[:, kti, :, 0:PD],
                    )
                    yield f'V{rt}'

            # ---- warmup: DMA + proj for sci 0,1,2; consts ----
            xraws = {s: dma_in(s) for s in range(3)}
            for sb_t, dr_t in [(wk, wk_d), (bk, bk_d), (wq, wq_d),
                               (bq, bq_d), (wv, wv_d), (bv, bv_d),
                               (wo, wo_d), (tri, tri_d),
                               (ones_col, onesc_d)]:
                nc.sync.dma_start(sb_t[:], dr_t[:])
            nc.gpsimd.memset(vts[:, :, :, PD:PD + 1], 1.0)
            nc.gpsimd.memset(vts8[:, :, :, PD:PD + 1], 1.0)
            # warmup handled through the queue: only T/Q/K of proj(0)
            # must precede the first attention group.

            # ---- flat cross-chunk pipelined attention ----
            from collections import deque
            units = [(j, g) for j in range(NQC) for g in range(2 * j + 2)]
            pending_norm = []     # deferred PE/DVE norm closures
            pending_outproj = deque()   # deferred per-b2 out-proj pieces
            pending_pv = None     # (j, g, pts) one group behind
            chunk_state = {}      # j -> (pvs, attnT)

            def issue_pv(j, g, pts):
                pvs = chunk_state[j][0]
                ngrp = 2 * j + 2
                kc0 = 2 * g
                if kc0 < 4 * j:
                    # far group: fp8 DoubleRow, both kc in one matmul
                    for h in range(2):
                        pt, _ = pts[h]
                        nc.tensor.matmul(
                            pvs[h][:, 0:QC],
                            vts8[:, kc0:kc0 + 2, h, 0:PD + 1],
                            pt[:, :, 0:QC],
                            start=(kc0 == 0), stop=False,
                            perf_mode=DR, skip_group_check=True,
                        )
                    return
                for i in range(2):
                    kc = kc0 + i
                    m = kc - 4 * j
                    cr = m * P
                    last = (kc == 2 * ngrp - 1)
                    for h in range(2):
                        pt, prs = pts[h]
                        # diagonal block from the masked pr tile
                        nc.tensor.matmul(
                            pvs[h][:, cr:cr + P],
                            vts[:, kc, h, :],
                            prs[i][:],
                            start=(kc == 0), stop=(last and m == 3),
                            skip_group_check=True,
                        )
                        if m < 3:
                            # unmasked remainder straight from pt
                            nc.tensor.matmul(
                                pvs[h][:, cr + P:QC],
                                vts[:, kc, h, :],
                                pt[:, i, cr + P:QC],
                                start=False, stop=last,
                                skip_group_check=True,
                            )

            def issue_norm_dve(j):
                """DVE part of normalization for chunk j (after last PV)."""
                pvs, attnT = chunk_state[j]
                den = p2s.tile([1, 2, QC], f32, tag="den", name=f"den{j}")
                for h in range(2):
                    nc.vector.tensor_copy(out=den[:, h, :],
                                          in_=pvs[h][PD:PD + 1, :])
                recsb = p2s.tile([1, 2, QC], f32, tag="rec")
                nc.vector.reciprocal_approx_fast(out=recsb[:], in_=den[:])
                recb = p2s.tile([1, 2, QC], bf16, tag="recb")
                # cast on ACT: shortens the DVE chain gating the deferred
                # bcp broadcast (tail flush already does this)
                nc.scalar.copy(out=recb[:], in_=recsb[:])

                def normB(pvs=pvs, attnT=attnT, recb=recb):
                    for h in range(2):
                        bcp = bps.tile([PD, QC], f32, tag="b", name="bcp")
                        nc.tensor.matmul(bcp[:], ones_col[:],
                                         recb[:, h, :],
                                         start=True, stop=True)
                        bcs = p2s.tile([PD, QC], f32, tag=f"bcs{h}",
                                       name="bcs")
                        nc.vector.tensor_copy(out=bcs[:], in_=bcp[:])
                        nc.vector.tensor_mul(
                            out=attnT[h * PD:(h + 1) * PD, :],
                            in0=pvs[h][0:PD, :], in1=bcs[:])
                pending_norm.append(normB)

                def outproj(b2, j=j, attnT=attnT):
                    psf = bps.tile([P, D], f32, tag="b",
                                   name=f"psf{j}_{b2}")
                    nc.tensor.matmul(
                        psf[:],
                        attnT[:, b2 * P:(b2 + 1) * P],
                        wo[:],
                        start=True, stop=True,
                    )
                    osb = p2s.tile([P, D], f32, tag="osb")
                    nc.vector.tensor_copy(out=osb[:], in_=psf[:])
                    dq = (nc.sync, nc.gpsimd, nc.sync, nc.gpsimd)[b2]
                    dq.dma_start(
                        out_d[j * QC + b2 * P:j * QC + (b2 + 1) * P, :],
                        osb[:],
                    )
                for b2 in range(QC // P):
                    pending_outproj.append(
                        (lambda b2=b2: outproj(b2)))

            proj_q = deque()          # (sci, generator)
            proj_done = 1             # all sci < proj_done fully issued

            _DONE = object()
            _RANK = {'T': 0, 'Q': 1, 'K': 2,
                     'V0': 3, 'V1': 4, 'V2': 5, 'V3': 6}
            proj_prog = {}            # sci -> highest stage rank issued

            def pull_proj(n=1):
                nonlocal proj_done
                for _ in range(n):
                    while proj_q:
                        sci, it = proj_q[0]
                        lab = next(it, _DONE)
                        if lab is _DONE:
                            proj_q.popleft()
                            proj_done = sci + 1
                            continue
                        proj_prog[sci] = _RANK[lab]
                        return

            def ensure_proj(sci, stage='V3'):
                r = _RANK[stage]
                while proj_prog.get(sci, -1) < r and proj_q:
                    pull_proj()

            proj_q.append((0, proj_steps(0, xraws[0], bps)))
            proj_q.append((1, proj_steps(1, xraws[1], bps)))
            ensure_proj(0, 'K')
            for j, g in units:
                if g == 0:
                    # chunk start: prefetch DMA, enqueue proj of sci j+2
                    # (pv/attnT allocated lazily at first PV write, so the
                    # WAR snapshot sees every reader of the old buffers)
                    if j + 3 < NSC:
                        xraws[j + 3] = dma_in(j + 3)
                    if j + 2 < NSC:
                        proj_q.append(
                            (j + 2, proj_steps(j + 2, xraws[j + 2], bps)))
                # deps are program-order: Q of this chunk and K of every
                # kc this group touches must already be issued; V is
                # ensured at PV-issue time (one group later)
                ensure_proj(j, 'Q')
                ensure_proj((2 * g + 1) // 4, 'K')
                kc0 = 2 * g
                m0 = kc0 - 4 * j
                crg = max(0, m0) * P
                qcol0 = j * QC
                pts = {}
                band = (kc0 >= 4 * j)
                ptdt = bf16 if band else fp8
                for h in range(2):
                    qk = qkps.tile([P, 2, QC], f32, tag=f"qk{h}",
                                   name=f"qk{j}_{g}_{h}")
                    po = h * PD
                    for i in range(2):
                        kc = kc0 + i
                        nc.tensor.matmul(
                            qk[:, i, crg:QC],
                            kts[po:po + PD, kc * P:(kc + 1) * P],
                            qt[po:po + PD, qcol0 + crg:qcol0 + QC],
                            start=True, stop=True,
                        )
                    pt = p2s.tile([P, 2, QC], ptdt,
                                  tag=f"pt{'b' if band else 'f'}{h}",
                                  name=f"pt{j}_{g}_{h}")
                    nc.scalar.activation(pt[:, :, crg:QC],
                                         qk[:, :, crg:QC],
                                         Exp, scale=ESCALE)
                    prs = {}
                    for i in range(2):
                        m = kc0 + i - 4 * j
                        if 0 <= m < 4:
                            mc = m * P
                            pr = p2s.tile([P, P], bf16, tag=f"pr{h}{i}",
                                          name=f"pr{j}_{g}_{h}")
                            nc.vector.tensor_mul(
                                out=pr[:], in0=pt[:, i, mc:mc + P],
                                in1=tri[:])
                            prs[i] = pr
                    pts[h] = (pt, prs)
                if pending_norm and g == 1:
                    # must flush before this iteration's issue_pv writes
                    # the rotating pv/attnT buffers the closures read
                    for fn in pending_norm:
                        fn()
                    pending_norm = []
                if pending_outproj:
                    pending_outproj.popleft()()
                if pending_pv is not None:
                    pj, pg, ppts = pending_pv
                    kchi = 2 * pg + 1
                    ensure_proj(kchi // 4, f'V{kchi % 4}')
                    if pj not in chunk_state:
                        chunk_state[pj] = (
                            {h: pvps.tile([PD + 1, QC], f32,
                                          tag=f"pv{h}", name=f"pv{pj}_{h}")
                             for h in range(2)},
                            p2.tile([P, QC], bf16, tag="attnT",
                                    name=f"attnT{pj}"),
                        )
                    issue_pv(pj, pg, ppts)
                    if pg == 2 * pj + 1:      # was last group of chunk pj
                        issue_norm_dve(pj)
                pending_pv = (j, g, pts)
                pull_proj(4 if j <= 4 else 2)

            pj, pg, ppts = pending_pv
            ensure_proj((2 * pg + 1) // 4, 'V3')
            issue_pv(pj, pg, ppts)
            # flush chunk 6's deferred norm/outproj first so its PE work
            # overlaps chunk 7's tail chain
            for fn in pending_norm:
                fn()
            pending_norm = []
            while pending_outproj:
                pending_outproj.popleft()()
            # pipelined tail for the last chunk: per-head norm chains,
            # recb/bcs casts on the (idle) ACT engine
            pvs7, attnT7 = chunk_state[pj]
            recbs = {}
            for h in range(2):
                den = p2s.tile([1, QC], f32, tag=f"dent{h}")
                nc.vector.tensor_copy(out=den[:], in_=pvs7[h][PD:PD + 1, :])
                rec = p2s.tile([1, QC], f32, tag=f"rect{h}")
                nc.vector.reciprocal_approx_fast(out=rec[:], in_=den[:])
                recb = p2s.tile([1, QC], bf16, tag=f"recbt{h}")
                nc.scalar.copy(out=recb[:], in_=rec[:])
                recbs[h] = recb
            for h in range(2):
                bcp = bps.tile([PD, QC], f32, tag="b", name=f"bcpt{h}")
                nc.tensor.matmul(bcp[:], ones_col[:], recbs[h][:],
                                 start=True, stop=True)
                bcs = p2s.tile([PD, QC], f32, tag=f"bcst{h}")
                nc.scalar.copy(out=bcs[:], in_=bcp[:])
                nc.vector.tensor_mul(
                    out=attnT7[h * PD:(h + 1) * PD, :],
                    in0=pvs7[h][0:PD, :], in1=bcs[:])
            for b2 in range(QC // P):
                psf = bps.tile([P, D], f32, tag="b", name=f"psft{b2}")
                nc.tensor.matmul(psf[:], attnT7[:, b2 * P:(b2 + 1) * P],
                                 wo[:], start=True, stop=True)
                osb = p2s.tile([P, D], f32, tag="osb")
                if b2 % 2 == 0:
                    nc.vector.tensor_copy(out=osb[:], in_=psf[:])
                else:
                    nc.scalar.copy(out=osb[:], in_=psf[:])
                dq = (nc.sync, nc.gpsimd, nc.sync, nc.gpsimd)[b2]
                dq.dma_start(
                    out_d[pj * QC + b2 * P:pj * QC + (b2 + 1) * P, :],
                    osb[:],
                )
    nc.finalize()
    return nc


def _build_generic(mode: str):
    """Fallback build for non-causal masks (none / binary / additive).
    Query-sharded: core c handles batch c//4, query rows (c%4)*QR.."""
    import concourse.mybir as mybir
    import concourse.tile as tile
    from concourse import bacc
    from concourse.masks import make_identity

    f32 = mybir.dt.float32
    f32r = mybir.dt.float32r
    bf16 = mybir.dt.bfloat16
    Exp = mybir.ActivationFunctionType.Exp
    Alu = mybir.AluOpType

    GQC = 512
    GNQC = QR // GQC
    NKTg = S // P
    HG = 4
    NHG = H // HG
    HPg = H // 2

    nc = bacc.Bacc(debug=False, target_bir_lowering=False)

    xb = nc.declare_dram_parameter("xb", [S, D], f32, isOutput=False)
    xq = nc.declare_dram_parameter("xq", [QR, D], f32, isOutput=False)
    wq_d = nc.declare_dram_parameter("wq", [P, DC, D], f32r, isOutput=False)
    wk_d = nc.declare_dram_parameter("wk", [P, DC, D], f32r, isOutput=False)
    wv_d = nc.declare_dram_parameter("wv", [P, DC, D], f32r, isOutput=False)
    wo_d = nc.declare_dram_parameter("wo", [P, DC, D], f32r, isOutput=False)
    bq_d = nc.declare_dram_parameter("bq", [P, DC], f32, isOutput=False)
    bk_d = nc.declare_dram_parameter("bk", [P, DC], f32, isOutput=False)
    bv_d = nc.declare_dram_parameter("bv", [P, D], f32, isOutput=False)
    bo_d = nc.declare_dram_parameter("bo", [P, D], f32, isOutput=False)
    ones_d = nc.declare_dram_parameter("ones", [P, H], bf16, isOutput=False)
    onesr_d = nc.declare_dram_parameter("onesr", [1, PD], f32r, isOutput=False)
    if mode == "add":
        maskT_d = nc.declare_dram_parameter("maskT", [S, QR], f32, isOutput=False)
    elif mode == "bin":
        maskT_d = nc.declare_dram_parameter("maskT", [S, QR], bf16, isOutput=False)
    out_d = nc.declare_dram_parameter("out", [QR, D], f32, isOutput=True)

    with tile.TileContext(nc) as tc, nc.allow_low_precision(
            reason="float32r tiles are 4-byte fp32; PE rounds reads only"):
        with (
            tc.tile_pool(name="const", bufs=1) as constp,
            tc.tile_pool(name="kt", bufs=1) as ktp,
            tc.tile_pool(name="vt", bufs=1) as vtp,
            tc.tile_pool(name="work", bufs=3) as work,
        ):
            ident = constp.tile([P, P], f32, tag="ident")
            make_identity(nc, ident)
            ones_col = constp.tile([1, PD], f32r, tag="ones")
            nc.sync.dma_start(ones_col[:], onesr_d[:])

            wq = constp.tile([P, DC, D], f32r, tag="wq")
            wo = constp.tile([P, DC, D], f32r, tag="wo")
            bq = constp.tile([P, DC], f32, tag="bq")
            bo = constp.tile([P, D], f32, tag="bo")
            for sb_t, dr_t in [(wq, wq_d), (wo, wo_d), (bq, bq_d), (bo, bo_d)]:
                nc.sync.dma_start(sb_t[:], dr_t[:])

            kts = [ktp.tile([P, HPg, SC], bf16, tag=f"kt{i}", name=f"kt{i}")
                   for i in range(NSC)]
            vts = [vtp.tile([P, H, PD + 1], bf16, tag=f"v{i}", name=f"v{i}")
                   for i in range(NKTg)]
            for t in vts:
                nc.sync.dma_start(t[:, :, PD:PD + 1], ones_d[:, :, None])

            with (
                tc.tile_pool(name="p1w", bufs=1) as p1w,
                tc.tile_pool(name="ps1", bufs=2, space="PSUM") as ps1,
            ):
                wk = p1w.tile([P, DC, D], f32r, tag="wk")
                wv = p1w.tile([P, DC, D], f32r, tag="wv")
                bk = p1w.tile([P, DC], f32, tag="bk")
                bv = p1w.tile([P, D], f32, tag="bv")
                for sb_t, dr_t in [(wk, wk_d), (wv, wv_d), (bk, bk_d), (bv, bv_d)]:
                    nc.sync.dma_start(sb_t[:], dr_t[:])

                for sci in range(NSC):
                    xraw = work.tile([P, SC // P, D], f32, tag="xraw")
                    nc.sync.dma_start(
                        xraw[:],
                        xb[sci * SC:(sci + 1) * SC].rearrange(
                            "(rt p) d -> p rt d", p=P),
                    )
                    xt = work.tile([P, DC, SC], f32r, tag="xt")
                    for rt in range(SC // P):
                        pst = ps1.tile([P, D], f32, tag="tps")
                        for dc in range(DC):
                            nc.tensor.transpose(
                                pst[:, dc * P:(dc + 1) * P],
                                xraw[:, rt, dc * P:(dc + 1) * P],
                                ident[:],
                            )
                        nc.scalar.copy(
                            out=xt[:, :, rt * P:(rt + 1) * P],
                            in_=pst[:].rearrange("p (dc j) -> p dc j", dc=DC),
                        )
                    for hp in range(HPg):
                        psk = ps1.tile([P, SC], f32, tag="kproj")
                        for dc in range(DC):
                            nc.tensor.matmul(
                                psk[:],
                                wk[:, dc, hp * P:(hp + 1) * P],
                                xt[:, dc, :],
                                start=(dc == 0), stop=(dc == DC - 1),
                            )
                        nc.scalar.add(kts[sci][:, hp, :], psk[:], bk[:, hp:hp + 1])
                    for rt in range(SC // P):
                        psv = ps1.tile([P, D], f32, tag="vproj")
                        for dc in range(DC):
                            nc.tensor.matmul(
                                psv[:],
                                xt[:, dc, rt * P:(rt + 1) * P],
                                wv[:, dc, :],
                                start=(dc == 0), stop=(dc == DC - 1),
                            )
                        kti = sci * (SC // P) + rt
                        nc.vector.tensor_add(
                            out=vts[kti][:, :, 0:PD],
                            in0=psv[:].rearrange("p (h d) -> p h d", h=H),
                            in1=bv[:].rearrange("p (h d) -> p h d", h=H),
                        )

            with (
                tc.tile_pool(name="p2", bufs=3) as p2,
                tc.tile_pool(name="p2s", bufs=2) as p2s,
                tc.tile_pool(name="p2a", bufs=1) as p2a,
                tc.tile_pool(name="qkps", bufs=3, space="PSUM") as qkps,
                tc.tile_pool(name="pvps", bufs=1, space="PSUM") as pvps,
                tc.tile_pool(name="fps", bufs=1, space="PSUM") as fps,
            ):
                for qc in range(GNQC):
                    xqraw = work.tile([P, GQC // P, D], f32, tag="xraw")
                    nc.sync.dma_start(
                        xqraw[:],
                        xq[qc * GQC:(qc + 1) * GQC].rearrange(
                            "(rt p) d -> p rt d", p=P),
                    )
                    xqt = work.tile([P, DC, GQC], f32r, tag="xt")
                    for rt in range(GQC // P):
                        pst = qkps.tile([P, D], f32, tag="qk")
                        for dc in range(DC):
                            nc.tensor.transpose(
                                pst[:, dc * P:(dc + 1) * P],
                                xqraw[:, rt, dc * P:(dc + 1) * P],
                                ident[:],
                            )
                        nc.scalar.copy(
                            out=xqt[:, :, rt * P:(rt + 1) * P],
                            in_=pst[:].rearrange("p (dc j) -> p dc j", dc=DC),
                        )
                    qt = p2.tile([P, HPg, GQC], bf16, tag="qt")
                    for hp in range(HPg):
                        psq = qkps.tile([P, D], f32, tag="qk")
                        for dc in range(DC):
                            nc.tensor.matmul(
                                psq[:, 0:GQC],
                                wq[:, dc, hp * P:(hp + 1) * P],
                                xqt[:, dc, :],
                                start=(dc == 0), stop=(dc == DC - 1),
                            )
                        nc.scalar.add(qt[:, hp, :], psq[:, 0:GQC], bq[:, hp:hp + 1])

                    attnT = p2a.tile([P, DC, GQC], f32r, tag="attnT")
                    for hg in range(NHG):
                        heads = range(hg * HG, (hg + 1) * HG)
                        pvs = {h: pvps.tile([PD + 1, GQC], f32, tag=f"pv{h % HG}",
                                            name=f"pv{h}")
                               for h in heads}
                        for kc in range(NKTg):
                            if mode == "add":
                                mt = p2s.tile([P, GQC], f32, tag="mt")
                            elif mode == "bin":
                                mt = p2s.tile([P, GQC], bf16, tag="mt")
                            if mode != "none":
                                nc.sync.dma_start(
                                    mt[:],
                                    maskT_d[kc * P:(kc + 1) * P,
                                            qc * GQC:(qc + 1) * GQC],
                                )
                            for h in heads:
                                po = (h % 2) * PD
                                pss = qkps.tile([P, D], f32, tag="qk")
                                nc.tensor.matmul(
                                    pss[:, 0:GQC],
                                    kts[kc // (SC // P)][
                                        po:po + PD, h // 2,
                                        (kc % (SC // P)) * P:
                                        (kc % (SC // P) + 1) * P],
                                    qt[po:po + PD, h // 2, :],
                                    start=True, stop=True,
                                )
                                pt = p2s.tile([P, GQC], bf16, tag="pt")
                                if mode == "add":
                                    st = p2s.tile([P, GQC], f32, tag="st")
                                    nc.vector.scalar_tensor_tensor(
                                        out=st[:], in0=mt[:], scalar=-1e9,
                                        in1=pss[:, 0:GQC],
                                        op0=Alu.mult, op1=Alu.add,
                                    )
                                    nc.scalar.activation(pt[:], st[:], Exp,
                                                         scale=0.125)
                                elif mode == "bin":
                                    pr = p2s.tile([P, GQC], bf16, tag="pr")
                                    nc.scalar.activation(pr[:], pss[:, 0:GQC], Exp,
                                                         scale=0.125)
                                    nc.vector.tensor_mul(
                                        out=pt[:], in0=pr[:], in1=mt[:])
                                else:
                                    nc.scalar.activation(pt[:], pss[:, 0:GQC], Exp,
                                                         scale=0.125)
                                nc.tensor.matmul(
                                    pvs[h][:],
                                    vts[kc][:, h, :],
                                    pt[:],
                                    start=(kc == 0), stop=(kc == NKTg - 1),
                                    skip_group_check=True,
                                )
                        for h in heads:
                            recip = p2s.tile([1, GQC], f32r, tag="recip")
                            nc.vector.reciprocal(recip[:], pvs[h][PD:PD + 1, :])
                            bcp = fps.tile([PD, GQC], f32, tag="fin")
                            nc.tensor.matmul(
                                bcp[:], ones_col[:], recip[:],
                                start=True, stop=True,
                            )
                            bcs = p2s.tile([PD, GQC], f32, tag="bcs")
                            nc.vector.tensor_copy(out=bcs[:], in_=bcp[:])
                            po = (h % 2) * PD
                            nc.vector.tensor_mul(
                                out=attnT[po:po + PD, h // 2, :],
                                in0=pvs[h][0:PD, :],
                                in1=bcs[:],
                            )

                    for rt in range(GQC // P):
                        psf = fps.tile([P, D], f32, tag="fin")
                        for dc in range(DC):
                            nc.tensor.matmul(
                                psf[:],
                                attnT[:, dc, rt * P:(rt + 1) * P],
                                wo[:, dc, :],
                                start=(dc == 0), stop=(dc == DC - 1),
                            )
                        osb = p2s.tile([P, D], f32, tag="osb")
                        nc.vector.tensor_add(out=osb[:], in0=psf[:], in1=bo[:])
                        nc.sync.dma_start(
                            out_d[qc * GQC + rt * P: qc * GQC + (rt + 1) * P, :],
                            osb[:],
                        )
    nc.finalize()
    return nc


def _get_prog(mode: str):
    if mode not in _prog_cache:
        _prog_cache[mode] = (_build_tril_v3() if mode == "tril"
                             else _build_generic(mode))
    return _prog_cache[mode]


def _q_rows(c, mode):
    r0 = (c % CPB) * QR
    return np.arange(r0, r0 + QR)


def _warr(W, dtype):
    return np.ascontiguousarray(
        np.asarray(W, dtype=np.float32).reshape(DC, P, D)
        .transpose(1, 0, 2)).astype(dtype)


def _barr(b):
    return np.ascontiguousarray(
        np.asarray(b, dtype=np.float32).reshape(DC, P).T)


def make_in_maps(inputs, mask, Wq, bq, Wk, bk, Wv, bv, Wo, bo):
    import ml_dtypes
    bf = ml_dtypes.bfloat16
    e4 = ml_dtypes.float8_e4m3
    inputs = np.asarray(inputs, dtype=np.float32)
    mask = np.asarray(mask, dtype=np.float32)
    _bo_stash["bo"] = np.asarray(bo, dtype=np.float32)
    if np.array_equal(mask, np.triu(np.ones((S, S), dtype=np.float32), 1)):
        mode = "tril"
    elif not np.any(mask):
        mode = "none"
    elif bool(((mask == 0.0) | (mask == 1.0)).all()):
        mode = "bin"
    else:
        mode = "add"

    in_maps = []
    if mode == "tril":
        Wq = np.asarray(Wq, np.float32)
        Wk = np.asarray(Wk, np.float32)
        Wv = np.asarray(Wv, np.float32)
        Wo = np.asarray(Wo, np.float32)
        bq16 = 16.0 * np.asarray(bq, np.float32)
        bk16 = 16.0 * np.asarray(bk, np.float32)
        bvf = np.asarray(bv, np.float32)
        tri = np.triu(np.ones((P, P), np.float32)).astype(bf)  # keep k <= q
        for c in range(NCORES):
            b = c // CPB
            hp = c % CPB
            cols = slice(hp * P, (hp + 1) * P)

            def wslice(W, dtype, scale=1.0):
                return np.ascontiguousarray(
                    (scale * W[:, cols]).reshape(DC, P, P)
                    .transpose(1, 0, 2)).astype(dtype)

            m = {
                "xb": np.ascontiguousarray(inputs[b]),
                "wq": wslice(Wq, e4, 16.0),
                "wk": wslice(Wk, e4, 16.0),
                "wv": wslice(Wv, bf),
                "wo": np.ascontiguousarray(Wo[cols, :]).astype(bf),
                "bq": np.ascontiguousarray(np.broadcast_to(
                    bq16[cols][:, None], (P, SC))),
                "bk": np.ascontiguousarray(np.broadcast_to(
                    bk16[cols][:, None], (P, SC))),
                "bv": np.ascontiguousarray(np.broadcast_to(
                    bvf[cols].reshape(1, 2, PD), (P, 2, PD))),
                "tri": np.ascontiguousarray(tri),
                "onesc": np.ones((1, PD), dtype=bf),
            }
            in_maps.append(m)
        return mode, in_maps

    # generic path (fp32/f32r), query-sharded
    if mode == "none":
        maskT = None
    elif mode == "bin":
        maskT = np.ascontiguousarray(1.0 - mask.T).astype(bf)
    else:
        maskT = np.ascontiguousarray(mask.T)
    shared = {
        "wq": _warr(Wq, np.float32), "wk": _warr(Wk, np.float32),
        "wv": _warr(Wv, np.float32), "wo": _warr(Wo, np.float32),
        "bq": _barr(bq), "bk": _barr(bk),
        "bv": np.ascontiguousarray(
            np.broadcast_to(np.asarray(bv, dtype=np.float32), (P, D))),
        "bo": np.ascontiguousarray(
            np.broadcast_to(np.asarray(bo, dtype=np.float32), (P, D))),
    }
    for c in range(NCORES):
        b = c // CPB
        rows = _q_rows(c, mode)
        m = dict(shared)
        m["ones"] = np.ones((P, H), dtype=bf)
        m["onesr"] = np.ones((1, PD), dtype=np.float32)
        m["xb"] = np.ascontiguousarray(inputs[b])
        m["xq"] = np.ascontiguousarray(inputs[b][rows])
        if maskT is not None:
            m["maskT"] = np.ascontiguousarray(maskT[:, rows])
        in_maps.append(m)
    return mode, in_maps


def assemble(results, mode):
    out = np.empty((B, S, D), dtype=np.float32)
    if mode == "tril":
        bo = _bo_stash["bo"]
        for b in range(B):
            acc = results[b * CPB]["out"].astype(np.float32).copy()
            for hp in range(1, CPB):
                acc += results[b * CPB + hp]["out"]
            out[b] = acc + bo
        return out
    for c in range(NCORES):
        b = c // CPB
        out[b, _q_rows(c, mode)] = results[c]["out"]
    return out


def kernel(inputs, mask, Wq, bq, Wk, bk, Wv, bv, Wo, bo):
    from concourse.bass_utils import run_bass_kernel_spmd

    mode, in_maps = make_in_maps(inputs, mask, Wq, bq, Wk, bk, Wv, bv, Wo, bo)
    nc = _get_prog(mode)
    res = run_bass_kernel_spmd(nc, in_maps, core_ids=list(range(NCORES)))
    return assemble(res.results, mode)


# revision 14
# speedup vs baseline: 1.0202x; 1.0202x over previous
"""Multi-head attention (B=2, S=4096, D=512, H=8) on 8 trn2 NeuronCores.

v3 sharding: head-pair tensor parallel. Core c = (batch c//4, head-pair
c%4). Each core projects K^T/V/Q^T for only its 2 heads over the full
sequence (no replicated projection work), runs causal flash attention for
those heads over all 4096 queries, and applies its 128-row slice of Wo to
produce a PARTIAL output [S, D]. The host sums the 4 partials per batch
and adds bo (no on-device collectives).

Why this layout wins over query-sharding:
  - K/V/Q projection work per core drops 4x (was replicated across the
    4 cores of a batch).
  - Causal structure is exact per core (single triangular mask tile at
    the diagonal block) -- no cross-core band masks, no interleaving.
  - ACT-engine exp is the critical path (~150us/core): exp instructions
    cover TWO kc blocks per issue via a bank-aligned 3D PSUM AP
    [128, 2, 512] (measured: one 2-bank exp = 1113ns vs 2x687ns).
  - Q/K projections run as fp8e4 DoubleRow matmuls (2 k-tiles per
    instruction, 2x throughput; weights prescaled x16 so fp8 range is
    used, compensated in the exp scale 0.125/256). V/Wo stay bf16:
    numpy simulation shows fp8 V/Wo break the 2e-2 rel-err budget
    (early causal queries average too few keys to wash out fp8 noise).
  - QK matmuls for the two heads sit at PE partition offsets 0/64 and
    are issued adjacently so the row-tiled PE can overlap them.
  - Projections stream through a global work queue interleaved into the
    attention groups, paced by ranked stage labels (T/Q/K/V0-V3) so each
    QK/PV reads only data whose writes are already issued (tile-framework
    deps are program-order), while filling PE bubbles under the ACT exp
    shadow. Attention starts as soon as proj(0) reaches its K stage.

PSUM budget (8 banks): qk scores [128,2,512]f32 x2 heads = 4, pv
accumulators [65,512]f32 x2 = 2, misc rotating [128,512]f32 x2 = 2
(transpose/proj/out-proj/broadcast all share the misc pool).

Hard-won HW constraints honored (sim passes but HW fails if not):
  - ACT APs may span PSUM banks only with bank-aligned 3D APs (runs must
    not cross a bank boundary),
  - DoubleRow ldweights tile stride must be a multiple of 16 bytes,
  - GPSIMD (Pool) cannot touch PSUM; PSUM drains go through DVE,
  - DVE ops must not read and write the same SBUF range in-place,
  - two matmuls must not write disjoint ranges of one PSUM bank.
"""

import numpy as np

# Problem dims (hardcoded per contract)
B, S, D, H, PD = 2, 4096, 512, 8, 64
P = 128
NCORES = 8
CPB = 4            # cores per batch (= head-pairs)
QC = 512           # attention q-chunk width
NQC = S // QC      # 8 chunks
SC = 512           # projection sequence chunk
NSC = S // SC      # 8
NKT = S // P       # 32 key subblocks of 128
DC = D // P        # 4 d-chunks of 128
HP = H // 2        # 4 head-pairs
QR = S // CPB      # (generic path) query rows per core

_prog_cache = {}
_bo_stash = {}


def _build_tril_v3():
    import concourse.mybir as mybir
    import concourse.tile as tile
    from concourse import bacc
    from concourse.masks import make_identity

    f32 = mybir.dt.float32
    bf16 = mybir.dt.bfloat16
    fp8 = mybir.dt.float8e4
    Exp = mybir.ActivationFunctionType.Exp
    DR = mybir.MatmulPerfMode.DoubleRow
    Alu = mybir.AluOpType
    ESCALE = 0.125 / 256.0   # 1/sqrt(64), /16^2 for prescaled Wq/Wk

    nc = bacc.Bacc(debug=False, target_bir_lowering=False)

    xb_d = nc.declare_dram_parameter("xb", [S, D], f32, isOutput=False)
    wq_d = nc.declare_dram_parameter("wq", [P, DC, P], fp8, isOutput=False)
    wk_d = nc.declare_dram_parameter("wk", [P, DC, P], fp8, isOutput=False)
    wv_d = nc.declare_dram_parameter("wv", [P, DC, P], bf16, isOutput=False)
    wo_d = nc.declare_dram_parameter("wo", [P, D], bf16, isOutput=False)
    bq_d = nc.declare_dram_parameter("bq", [P, SC], f32, isOutput=False)
    bk_d = nc.declare_dram_parameter("bk", [P, SC], f32, isOutput=False)
    bv_d = nc.declare_dram_parameter("bv", [P, 2, PD], f32, isOutput=False)
    tri_d = nc.declare_dram_parameter("tri", [P, P], bf16, isOutput=False)
    onesc_d = nc.declare_dram_parameter("onesc", [1, PD], bf16, isOutput=False)
    out_d = nc.declare_dram_parameter("out", [S, D], f32, isOutput=True)

    with tile.TileContext(nc) as tc, nc.allow_low_precision(
            reason="bf16/fp8 matmul operands; fp32 PSUM accumulation"):
        with (
            tc.tile_pool(name="const", bufs=1) as constp,
            tc.tile_pool(name="big", bufs=1) as bigp,
            tc.tile_pool(name="work", bufs=4) as work,
            tc.tile_pool(name="p2", bufs=3) as p2,
            tc.tile_pool(name="p2s", bufs=2) as p2s,
            tc.tile_pool(name="qkps", bufs=1, space="PSUM") as qkps,
            tc.tile_pool(name="pvps", bufs=1, space="PSUM") as pvps,
            tc.tile_pool(name="bps", bufs=2, space="PSUM") as bps,
        ):
            ident = constp.tile([P, P], f32, tag="ident")
            make_identity(nc, ident)
            identb = constp.tile([P, P], bf16, tag="identb")
            nc.vector.tensor_copy(out=identb[:], in_=ident[:])
            tri = constp.tile([P, P], bf16, tag="tri")
            ones_col = constp.tile([1, PD], bf16, tag="onesc")
            wq = constp.tile([P, DC, P], fp8, tag="wq")
            wk = constp.tile([P, DC, P], fp8, tag="wk")
            wv = constp.tile([P, DC, P], bf16, tag="wv")
            wo = constp.tile([P, D], bf16, tag="wo")
            bq = constp.tile([P, SC], f32, tag="bq")
            bk = constp.tile([P, SC], f32, tag="bk")
            bv = constp.tile([P, 2, PD], f32, tag="bv")

            # K^T/Q^T: partitions 0-63 = head0 dims, 64-127 = head1 dims
            kts = bigp.tile([P, S], bf16, tag="kts")
            qt = bigp.tile([P, S], bf16, tag="qt")
            # V rows + ones column per (kc, head); bf16 for diagonal
            # blocks (exact), fp8 for far blocks (DoubleRow PV)
            vts = bigp.tile([P, NKT, 2, PD + 1], bf16, tag="vts")
            vts8 = bigp.tile([P, NKT, 2, 72], fp8, tag="vts8")

            def dma_in(sci):
                # gpsimd queue supports casting DMAs: f32 DRAM -> bf16 SBUF
                xraw = work.tile([P, SC // P, D], bf16, tag="xraw",
                                 name=f"xraw{sci}")
                nc.gpsimd.dma_start(
                    xraw[:],
                    xb_d[sci * SC:(sci + 1) * SC].rearrange(
                        "(rt p) d -> p rt d", p=P),
                )
                return xraw

            def proj_steps(sci, xraw, pool):
                """Generator: projection of X chunk sci; yields stage
                labels as pieces complete (T/Q/K/V)."""
                xt8 = work.tile([P, DC, SC], fp8, tag="xt8",
                                name=f"xt8{sci}")
                xtb = work.tile([P, DC, SC], bf16, tag="xtb",
                                name=f"xtb{sci}")
                # transpose pieces (bf16: 1 cycle/row on the PE)
                for half in range(2):
                    for rt in range(2 * half, 2 * half + 2):
                        pst = pool.tile([P, D], bf16, tag="b",
                                        name=f"pst{sci}_{rt}")
                        for dc in range(DC):
                            nc.tensor.transpose(
                                pst[:, dc * P:(dc + 1) * P],
                                xraw[:, rt, dc * P:(dc + 1) * P],
                                identb[:],
                            )
                        nc.vector.tensor_copy(
                            out=xtb[:, :, rt * P:(rt + 1) * P],
                            in_=pst[:].rearrange("p (dc j) -> p dc j", dc=DC),
                        )
                        # fp8 casts alternate Pool/DVE: Pool's ~1.9us
                        # per cast otherwise serializes 7.5us ahead of
                        # the K/Q DoubleRow matmuls in forced bundles
                        ceng = nc.gpsimd if rt % 2 == 0 else nc.vector
                        ceng.tensor_copy(
                            out=xt8[:, :, rt * P:(rt + 1) * P],
                            in_=xtb[:, :, rt * P:(rt + 1) * P],
                        )
                    yield 'T'
                # Q^T first (chunk j's attention needs qt(j) before kts(j)),
                # then K^T, via fp8 DoubleRow
                for lab, w8, bias, dst in (('Q', wq, bq, qt),
                                           ('K', wk, bk, kts)):
                    psx = pool.tile([P, D], f32, tag="b", name=f"ps{sci}")
                    for di in range(2):
                        nc.tensor.matmul(
                            psx[:, 0:SC],
                            w8[:, 2 * di:2 * di + 2, :],
                            xt8[:, 2 * di:2 * di + 2, :],
                            start=(di == 0), stop=(di == 1),
                            perf_mode=DR,
                        )
                    # drains stay off ACT: an in-queue IDENTITY blocks
                    # later exps on psx completion (head-of-line)
                    nc.vector.tensor_add(
                        out=dst[:, sci * SC:(sci + 1) * SC],
                        in0=psx[:, 0:SC], in1=bias[:])
                    yield lab
                # V (bf16, exact) per row-tile
                for rt in range(SC // P):
                    psv = pool.tile([P, P], f32, tag="b",
                                    name=f"psv{sci}_{rt}")
                    for dc in range(DC):
                        nc.tensor.matmul(
                            psv[:],
                            xtb[:, dc, rt * P:(rt + 1) * P],
                            wv[:, dc, :],
                            start=(dc == 0), stop=(dc == DC - 1),
                        )
                    kti = sci * (SC // P) + rt
                    nc.vector.tensor_add(
                        out=vts[:, kti, :, 0:PD],
                        in0=psv[:].rearrange("p (h d) -> p h d", h=2),
                        in1=bv[:],
                    )
                    nc.gpsimd.tensor_copy(
                        out=vts8[:, kti, :, 0:PD],
                        in_=vts[:, kti, :, 0:PD],
                    )
                    yield f'V{rt}'

            # ---- warmup: DMA + proj for sci 0,1,2; consts ----
            xraws = {s: dma_in(s) for s in range(3)}
            for sb_t, dr_t in [(wk, wk_d), (bk, bk_d), (wq, wq_d),
                               (bq, bq_d), (wv, wv_d), (bv, bv_d),
                               (wo, wo_d), (tri, tri_d),
                               (ones_col, onesc_d)]:
                nc.sync.dma_start(sb_t[:], dr_t[:])
            nc.gpsimd.memset(vts[:, :, :, PD:PD + 1], 1.0)
            nc.gpsimd.memset(vts8[:, :, :, PD:PD + 1], 1.0)
            # warmup handled through the queue: only T/Q/K of proj(0)
            # must precede the first attention group.

            # ---- flat cross-chunk pipelined attention ----
            from collections import deque
            units = [(j, g) for j in range(NQC) for g in range(2 * j + 2)]
            pending_norm = []     # deferred PE/DVE norm closures
            pending_outproj = deque()   # deferred per-b2 out-proj pieces
            pending_pv = None     # (j, g, pts) one group behind
            chunk_state = {}      # j -> (pvs, attnT)

            def issue_pv(j, g, pts):
                pvs = chunk_state[j][0]
                ngrp = 2 * j + 2
                kc0 = 2 * g
                if kc0 < 4 * j:
                    # far group: fp8 DoubleRow, both kc in one matmul
                    for h in range(2):
                        pt, _ = pts[h]
                        nc.tensor.matmul(
                            pvs[h][:, 0:QC],
                            vts8[:, kc0:kc0 + 2, h, 0:PD + 1],
                            pt[:, :, 0:QC],
                            start=(kc0 == 0), stop=False,
                            perf_mode=DR, skip_group_check=True,
                        )
                    return
                for i in range(2):
                    kc = kc0 + i
                    m = kc - 4 * j
                    cr = m * P
                    last = (kc == 2 * ngrp - 1)
                    for h in range(2):
                        pt, prs = pts[h]
                        # diagonal block from the masked pr tile
                        nc.tensor.matmul(
                            pvs[h][:, cr:cr + P],
                            vts[:, kc, h, :],
                            prs[i][:],
                            start=(kc == 0), stop=(last and m == 3),
                            skip_group_check=True,
                        )
                        if m < 3:
                            # unmasked remainder straight from pt
                            nc.tensor.matmul(
                                pvs[h][:, cr + P:QC],
                                vts[:, kc, h, :],
                                pt[:, i, cr + P:QC],
                                start=False, stop=last,
                                skip_group_check=True,
                            )

            def issue_norm_dve(j):
                """DVE part of normalization for chunk j (after last PV)."""
                pvs, attnT = chunk_state[j]
                den = p2s.tile([1, 2, QC], f32, tag="den", name=f"den{j}")
                for h in range(2):
                    # ACT drains the denominator row: the chunk-boundary
                    # DVE queue is the congested one
                    nc.scalar.copy(out=den[:, h, :],
                                   in_=pvs[h][PD:PD + 1, :])
                recsb = p2s.tile([1, 2, QC], f32, tag="rec")
                nc.vector.reciprocal_approx_fast(out=recsb[:], in_=den[:])
                recb = p2s.tile([1, 2, QC], bf16, tag="recb")
                # cast on ACT: shortens the DVE chain gating the deferred
                # bcp broadcast (tail flush already does this)
                nc.scalar.copy(out=recb[:], in_=recsb[:])

                def normB(pvs=pvs, attnT=attnT, recb=recb):
                    for h in range(2):
                        bcp = bps.tile([PD, QC], f32, tag="b", name="bcp")
                        nc.tensor.matmul(bcp[:], ones_col[:],
                                         recb[:, h, :],
                                         start=True, stop=True)
                        bcs = p2s.tile([PD, QC], f32, tag=f"bcs{h}",
                                       name="bcs")
                        nc.vector.tensor_copy(out=bcs[:], in_=bcp[:])
                        nc.vector.tensor_mul(
                            out=attnT[h * PD:(h + 1) * PD, :],
                            in0=pvs[h][0:PD, :], in1=bcs[:])
                pending_norm.append(normB)

                def outproj(b2, j=j, attnT=attnT):
                    psf = bps.tile([P, D], f32, tag="b",
                                   name=f"psf{j}_{b2}")
                    nc.tensor.matmul(
                        psf[:],
                        attnT[:, b2 * P:(b2 + 1) * P],
                        wo[:],
                        start=True, stop=True,
                    )
                    osb = p2s.tile([P, D], f32, tag="osb")
                    nc.vector.tensor_copy(out=osb[:], in_=psf[:])
                    dq = (nc.sync, nc.gpsimd, nc.sync, nc.gpsimd)[b2]
                    dq.dma_start(
                        out_d[j * QC + b2 * P:j * QC + (b2 + 1) * P, :],
                        osb[:],
                    )
                for b2 in range(QC // P):
                    pending_outproj.append(
                        (lambda b2=b2: outproj(b2)))

            proj_q = deque()          # (sci, generator)
            proj_done = 1             # all sci < proj_done fully issued

            _DONE = object()
            _RANK = {'T': 0, 'Q': 1, 'K': 2,
                     'V0': 3, 'V1': 4, 'V2': 5, 'V3': 6}
            proj_prog = {}            # sci -> highest stage rank issued

            def pull_proj(n=1):
                nonlocal proj_done
                for _ in range(n):
                    while proj_q:
                        sci, it = proj_q[0]
                        lab = next(it, _DONE)
                        if lab is _DONE:
                            proj_q.popleft()
                            proj_done = sci + 1
                            continue
                        proj_prog[sci] = _RANK[lab]
                        return

            def ensure_proj(sci, stage='V3'):
                r = _RANK[stage]
                while proj_prog.get(sci, -1) < r and proj_q:
                    pull_proj()

            proj_q.append((0, proj_steps(0, xraws[0], bps)))
            proj_q.append((1, proj_steps(1, xraws[1], bps)))
            ensure_proj(0, 'K')
            for j, g in units:
                if g == 0:
                    # chunk start: prefetch DMA, enqueue proj of sci j+2
                    # (pv/attnT allocated lazily at first PV write, so the
                    # WAR snapshot sees every reader of the old buffers)
                    if j + 3 < NSC:
                        xraws[j + 3] = dma_in(j + 3)
                    if j + 2 < NSC:
                        proj_q.append(
                            (j + 2, proj_steps(j + 2, xraws[j + 2], bps)))
                # deps are program-order: Q of this chunk and K of every
                # kc this group touches must already be issued; V is
                # ensured at PV-issue time (one group later)
                ensure_proj(j, 'Q')
                ensure_proj((2 * g + 1) // 4, 'K')
                kc0 = 2 * g
                m0 = kc0 - 4 * j
                crg = max(0, m0) * P
                qcol0 = j * QC
                pts = {}
                band = (kc0 >= 4 * j)
                ptdt = bf16 if band else fp8
                for h in range(2):
                    qk = qkps.tile([P, 2, QC], f32, tag=f"qk{h}",
                                   name=f"qk{j}_{g}_{h}")
                    po = h * PD
                    for i in range(2):
                        kc = kc0 + i
                        nc.tensor.matmul(
                            qk[:, i, crg:QC],
                            kts[po:po + PD, kc * P:(kc + 1) * P],
                            qt[po:po + PD, qcol0 + crg:qcol0 + QC],
                            start=True, stop=True,
                        )
                    pt = p2s.tile([P, 2, QC], ptdt,
                                  tag=f"pt{'b' if band else 'f'}{h}",
                                  name=f"pt{j}_{g}_{h}")
                    nc.scalar.activation(pt[:, :, crg:QC],
                                         qk[:, :, crg:QC],
                                         Exp, scale=ESCALE)
                    prs = {}
                    for i in range(2):
                        m = kc0 + i - 4 * j
                        if 0 <= m < 4:
                            mc = m * P
                            pr = p2s.tile([P, P], bf16, tag=f"pr{h}{i}",
                                          name=f"pr{j}_{g}_{h}")
                            nc.vector.tensor_mul(
                                out=pr[:], in0=pt[:, i, mc:mc + P],
                                in1=tri[:])
                            prs[i] = pr
                    pts[h] = (pt, prs)
                if pending_norm and g == 1:
                    # must flush before this iteration's issue_pv writes
                    # the rotating pv/attnT buffers the closures read
                    for fn in pending_norm:
                        fn()
                    pending_norm = []
                if pending_outproj:
                    pending_outproj.popleft()()
                if pending_pv is not None:
                    pj, pg, ppts = pending_pv
                    kchi = 2 * pg + 1
                    ensure_proj(kchi // 4, f'V{kchi % 4}')
                    if pj not in chunk_state:
                        chunk_state[pj] = (
                            {h: pvps.tile([PD + 1, QC], f32,
                                          tag=f"pv{h}", name=f"pv{pj}_{h}")
                             for h in range(2)},
                            p2.tile([P, QC], bf16, tag="attnT",
                                    name=f"attnT{pj}"),
                        )
                    issue_pv(pj, pg, ppts)
                    if pg == 2 * pj + 1:      # was last group of chunk pj
                        issue_norm_dve(pj)
                pending_pv = (j, g, pts)
                pull_proj(4 if j <= 4 else 2)

            pj, pg, ppts = pending_pv
            ensure_proj((2 * pg + 1) // 4, 'V3')
            issue_pv(pj, pg, ppts)
            # flush chunk 6's deferred norm/outproj first so its PE work
            # overlaps chunk 7's tail chain
            for fn in pending_norm:
                fn()
            pending_norm = []
            while pending_outproj:
                pending_outproj.popleft()()
            # pipelined tail for the last chunk: per-head norm chains,
            # recb/bcs casts on the (idle) ACT engine
            pvs7, attnT7 = chunk_state[pj]
            recbs = {}
            for h in range(2):
                den = p2s.tile([1, QC], f32, tag=f"dent{h}")
                nc.vector.tensor_copy(out=den[:], in_=pvs7[h][PD:PD + 1, :])
                rec = p2s.tile([1, QC], f32, tag=f"rect{h}")
                nc.vector.reciprocal_approx_fast(out=rec[:], in_=den[:])
                recb = p2s.tile([1, QC], bf16, tag=f"recbt{h}")
                nc.scalar.copy(out=recb[:], in_=rec[:])
                recbs[h] = recb
            for h in range(2):
                bcp = bps.tile([PD, QC], f32, tag="b", name=f"bcpt{h}")
                nc.tensor.matmul(bcp[:], ones_col[:], recbs[h][:],
                                 start=True, stop=True)
                bcs = p2s.tile([PD, QC], f32, tag=f"bcst{h}")
                nc.scalar.copy(out=bcs[:], in_=bcp[:])
                nc.vector.tensor_mul(
                    out=attnT7[h * PD:(h + 1) * PD, :],
                    in0=pvs7[h][0:PD, :], in1=bcs[:])
            for b2 in range(QC // P):
                psf = bps.tile([P, D], f32, tag="b", name=f"psft{b2}")
                nc.tensor.matmul(psf[:], attnT7[:, b2 * P:(b2 + 1) * P],
                                 wo[:], start=True, stop=True)
                osb = p2s.tile([P, D], f32, tag="osb")
                if b2 % 2 == 0:
                    nc.vector.tensor_copy(out=osb[:], in_=psf[:])
                else:
                    nc.scalar.copy(out=osb[:], in_=psf[:])
                dq = (nc.sync, nc.gpsimd, nc.sync, nc.gpsimd)[b2]
                dq.dma_start(
                    out_d[pj * QC + b2 * P:pj * QC + (b2 + 1) * P, :],
                    osb[:],
                )
    nc.finalize()
    return nc


def _build_generic(mode: str):
    """Fallback build for non-causal masks (none / binary / additive).
    Query-sharded: core c handles batch c//4, query rows (c%4)*QR.."""
    import concourse.mybir as mybir
    import concourse.tile as tile
    from concourse import bacc
    from concourse.masks import make_identity

    f32 = mybir.dt.float32
    f32r = mybir.dt.float32r
    bf16 = mybir.dt.bfloat16
    Exp = mybir.ActivationFunctionType.Exp
    Alu = mybir.AluOpType

    GQC = 512
    GNQC = QR // GQC
    NKTg = S // P
    HG = 4
    NHG = H // HG
    HPg = H // 2

    nc = bacc.Bacc(debug=False, target_bir_lowering=False)

    xb = nc.declare_dram_parameter("xb", [S, D], f32, isOutput=False)
    xq = nc.declare_dram_parameter("xq", [QR, D], f32, isOutput=False)
    wq_d = nc.declare_dram_parameter("wq", [P, DC, D], f32r, isOutput=False)
    wk_d = nc.declare_dram_parameter("wk", [P, DC, D], f32r, isOutput=False)
    wv_d = nc.declare_dram_parameter("wv", [P, DC, D], f32r, isOutput=False)
    wo_d = nc.declare_dram_parameter("wo", [P, DC, D], f32r, isOutput=False)
    bq_d = nc.declare_dram_parameter("bq", [P, DC], f32, isOutput=False)
    bk_d = nc.declare_dram_parameter("bk", [P, DC], f32, isOutput=False)
    bv_d = nc.declare_dram_parameter("bv", [P, D], f32, isOutput=False)
    bo_d = nc.declare_dram_parameter("bo", [P, D], f32, isOutput=False)
    ones_d = nc.declare_dram_parameter("ones", [P, H], bf16, isOutput=False)
    onesr_d = nc.declare_dram_parameter("onesr", [1, PD], f32r, isOutput=False)
    if mode == "add":
        maskT_d = nc.declare_dram_parameter("maskT", [S, QR], f32, isOutput=False)
    elif mode == "bin":
        maskT_d = nc.declare_dram_parameter("maskT", [S, QR], bf16, isOutput=False)
    out_d = nc.declare_dram_parameter("out", [QR, D], f32, isOutput=True)

    with tile.TileContext(nc) as tc, nc.allow_low_precision(
            reason="float32r tiles are 4-byte fp32; PE rounds reads only"):
        with (
            tc.tile_pool(name="const", bufs=1) as constp,
            tc.tile_pool(name="kt", bufs=1) as ktp,
            tc.tile_pool(name="vt", bufs=1) as vtp,
            tc.tile_pool(name="work", bufs=3) as work,
        ):
            ident = constp.tile([P, P], f32, tag="ident")
            make_identity(nc, ident)
            ones_col = constp.tile([1, PD], f32r, tag="ones")
            nc.sync.dma_start(ones_col[:], onesr_d[:])

            wq = constp.tile([P, DC, D], f32r, tag="wq")
            wo = constp.tile([P, DC, D], f32r, tag="wo")
            bq = constp.tile([P, DC], f32, tag="bq")
            bo = constp.tile([P, D], f32, tag="bo")
            for sb_t, dr_t in [(wq, wq_d), (wo, wo_d), (bq, bq_d), (bo, bo_d)]:
                nc.sync.dma_start(sb_t[:], dr_t[:])

            kts = [ktp.tile([P, HPg, SC], bf16, tag=f"kt{i}", name=f"kt{i}")
                   for i in range(NSC)]
            vts = [vtp.tile([P, H, PD + 1], bf16, tag=f"v{i}", name=f"v{i}")
                   for i in range(NKTg)]
            for t in vts:
                nc.sync.dma_start(t[:, :, PD:PD + 1], ones_d[:, :, None])

            with (
                tc.tile_pool(name="p1w", bufs=1) as p1w,
                tc.tile_pool(name="ps1", bufs=2, space="PSUM") as ps1,
            ):
                wk = p1w.tile([P, DC, D], f32r, tag="wk")
                wv = p1w.tile([P, DC, D], f32r, tag="wv")
                bk = p1w.tile([P, DC], f32, tag="bk")
                bv = p1w.tile([P, D], f32, tag="bv")
                for sb_t, dr_t in [(wk, wk_d), (wv, wv_d), (bk, bk_d), (bv, bv_d)]:
                    nc.sync.dma_start(sb_t[:], dr_t[:])

                for sci in range(NSC):
                    xraw = work.tile([P, SC // P, D], f32, tag="xraw")
                    nc.sync.dma_start(
                        xraw[:],
                        xb[sci * SC:(sci + 1) * SC].rearrange(
                            "(rt p) d -> p rt d", p=P),
                    )
                    xt = work.tile([P, DC, SC], f32r, tag="xt")
                    for rt in range(SC // P):
                        pst = ps1.tile([P, D], f32, tag="tps")
                        for dc in range(DC):
                            nc.tensor.transpose(
                                pst[:, dc * P:(dc + 1) * P],
                                xraw[:, rt, dc * P:(dc + 1) * P],
                                ident[:],
                            )
                        nc.scalar.copy(
                            out=xt[:, :, rt * P:(rt + 1) * P],
                            in_=pst[:].rearrange("p (dc j) -> p dc j", dc=DC),
                        )
                    for hp in range(HPg):
                        psk = ps1.tile([P, SC], f32, tag="kproj")
                        for dc in range(DC):
                            nc.tensor.matmul(
                                psk[:],
                                wk[:, dc, hp * P:(hp + 1) * P],
                                xt[:, dc, :],
                                start=(dc == 0), stop=(dc == DC - 1),
                            )
                        nc.scalar.add(kts[sci][:, hp, :], psk[:], bk[:, hp:hp + 1])
                    for rt in range(SC // P):
                        psv = ps1.tile([P, D], f32, tag="vproj")
                        for dc in range(DC):
                            nc.tensor.matmul(
                                psv[:],
                                xt[:, dc, rt * P:(rt + 1) * P],
                                wv[:, dc, :],
                                start=(dc == 0), stop=(dc == DC - 1),
                            )
                        kti = sci * (SC // P) + rt
                        nc.vector.tensor_add(
                            out=vts[kti][:, :, 0:PD],
                            in0=psv[:].rearrange("p (h d) -> p h d", h=H),
                            in1=bv[:].rearrange("p (h d) -> p h d", h=H),
                        )

            with (
                tc.tile_pool(name="p2", bufs=3) as p2,
                tc.tile_pool(name="p2s", bufs=2) as p2s,
                tc.tile_pool(name="p2a", bufs=1) as p2a,
                tc.tile_pool(name="qkps", bufs=3, space="PSUM") as qkps,
                tc.tile_pool(name="pvps", bufs=1, space="PSUM") as pvps,
                tc.tile_pool(name="fps", bufs=1, space="PSUM") as fps,
            ):
                for qc in range(GNQC):
                    xqraw = work.tile([P, GQC // P, D], f32, tag="xraw")
                    nc.sync.dma_start(
                        xqraw[:],
                        xq[qc * GQC:(qc + 1) * GQC].rearrange(
                            "(rt p) d -> p rt d", p=P),
                    )
                    xqt = work.tile([P, DC, GQC], f32r, tag="xt")
                    for rt in range(GQC // P):
                        pst = qkps.tile([P, D], f32, tag="qk")
                        for dc in range(DC):
                            nc.tensor.transpose(
                                pst[:, dc * P:(dc + 1) * P],
                                xqraw[:, rt, dc * P:(dc + 1) * P],
                                ident[:],
                            )
                        nc.scalar.copy(
                            out=xqt[:, :, rt * P:(rt + 1) * P],
                            in_=pst[:].rearrange("p (dc j) -> p dc j", dc=DC),
                        )
                    qt = p2.tile([P, HPg, GQC], bf16, tag="qt")
                    for hp in range(HPg):
                        psq = qkps.tile([P, D], f32, tag="qk")
                        for dc in range(DC):
                            nc.tensor.matmul(
                                psq[:, 0:GQC],
                                wq[:, dc, hp * P:(hp + 1) * P],
                                xqt[:, dc, :],
                                start=(dc == 0), stop=(dc == DC - 1),
                            )
                        nc.scalar.add(qt[:, hp, :], psq[:, 0:GQC], bq[:, hp:hp + 1])

                    attnT = p2a.tile([P, DC, GQC], f32r, tag="attnT")
                    for hg in range(NHG):
                        heads = range(hg * HG, (hg + 1) * HG)
                        pvs = {h: pvps.tile([PD + 1, GQC], f32, tag=f"pv{h % HG}",
                                            name=f"pv{h}")
                               for h in heads}
                        for kc in range(NKTg):
                            if mode == "add":
                                mt = p2s.tile([P, GQC], f32, tag="mt")
                            elif mode == "bin":
                                mt = p2s.tile([P, GQC], bf16, tag="mt")
                            if mode != "none":
                                nc.sync.dma_start(
                                    mt[:],
                                    maskT_d[kc * P:(kc + 1) * P,
                                            qc * GQC:(qc + 1) * GQC],
                                )
                            for h in heads:
                                po = (h % 2) * PD
                                pss = qkps.tile([P, D], f32, tag="qk")
                                nc.tensor.matmul(
                                    pss[:, 0:GQC],
                                    kts[kc // (SC // P)][
                                        po:po + PD, h // 2,
                                        (kc % (SC // P)) * P:
                                        (kc % (SC // P) + 1) * P],
                                    qt[po:po + PD, h // 2, :],
                                    start=True, stop=True,
                                )
                                pt = p2s.tile([P, GQC], bf16, tag="pt")
                                if mode == "add":
                                    st = p2s.tile([P, GQC], f32, tag="st")
                                    nc.vector.scalar_tensor_tensor(
                                        out=st[:], in0=mt[:], scalar=-1e9,
                                        in1=pss[:, 0:GQC],
                                        op0=Alu.mult, op1=Alu.add,
                                    )
                                    nc.scalar.activation(pt[:], st[:], Exp,
                                                         scale=0.125)
                                elif mode == "bin":
                                    pr = p2s.tile([P, GQC], bf16, tag="pr")
                                    nc.scalar.activation(pr[:], pss[:, 0:GQC], Exp,
                                                         scale=0.125)
                                    nc.vector.tensor_mul(
                                        out=pt[:], in0=pr[:], in1=mt[:])
                                else:
                                    nc.scalar.activation(pt[:], pss[:, 0:GQC], Exp,
                                                         scale=0.125)
                                nc.tensor.matmul(
                                    pvs[h][:],
                                    vts[kc][:, h, :],
                                    pt[:],
                                    start=(kc == 0), stop=(kc == NKTg - 1),
                                    skip_group_check=True,
                                )
                        for h in heads:
                            recip = p2s.tile([1, GQC], f32r, tag="recip")
                            nc.vector.reciprocal(recip[:], pvs[h][PD:PD + 1, :])
                            bcp = fps.tile([PD, GQC], f32, tag="fin")
                            nc.tensor.matmul(
                                bcp[:], ones_col[:], recip[:],
                                start=True, stop=True,
                            )
                            bcs = p2s.tile([PD, GQC], f32, tag="bcs")
                            nc.vector.tensor_copy(out=bcs[:], in_=bcp[:])
                            po = (h % 2) * PD
                            nc.vector.tensor_mul(
                                out=attnT[po:po + PD, h // 2, :],
                                in0=pvs[h][0:PD, :],
                                in1=bcs[:],
                            )

                    for rt in range(GQC // P):
                        psf = fps.tile([P, D], f32, tag="fin")
                        for dc in range(DC):
                            nc.tensor.matmul(
                                psf[:],
                                attnT[:, dc, rt * P:(rt + 1) * P],
                                wo[:, dc, :],
                                start=(dc == 0), stop=(dc == DC - 1),
                            )
                        osb = p2s.tile([P, D], f32, tag="osb")
                        nc.vector.tensor_add(out=osb[:], in0=psf[:], in1=bo[:])
                        nc.sync.dma_start(
                            out_d[qc * GQC + rt * P: qc * GQC + (rt + 1) * P, :],
                            osb[:],
                        )
    nc.finalize()
    return nc


def _get_prog(mode: str):
    if mode not in _prog_cache:
        _prog_cache[mode] = (_build_tril_v3() if mode == "tril"
                             else _build_generic(mode))
    return _prog_cache[mode]


def _q_rows(c, mode):
    r0 = (c % CPB) * QR
    return np.arange(r0, r0 + QR)


def _warr(W, dtype):
    return np.ascontiguousarray(
        np.asarray(W, dtype=np.float32).reshape(DC, P, D)
        .transpose(1, 0, 2)).astype(dtype)


def _barr(b):
    return np.ascontiguousarray(
        np.asarray(b, dtype=np.float32).reshape(DC, P).T)


def make_in_maps(inputs, mask, Wq, bq, Wk, bk, Wv, bv, Wo, bo):
    import ml_dtypes
    bf = ml_dtypes.bfloat16
    e4 = ml_dtypes.float8_e4m3
    inputs = np.asarray(inputs, dtype=np.float32)
    mask = np.asarray(mask, dtype=np.float32)
    _bo_stash["bo"] = np.asarray(bo, dtype=np.float32)
    if np.array_equal(mask, np.triu(np.ones((S, S), dtype=np.float32), 1)):
        mode = "tril"
    elif not np.any(mask):
        mode = "none"
    elif bool(((mask == 0.0) | (mask == 1.0)).all()):
        mode = "bin"
    else:
        mode = "add"

    in_maps = []
    if mode == "tril":
        Wq = np.asarray(Wq, np.float32)
        Wk = np.asarray(Wk, np.float32)
        Wv = np.asarray(Wv, np.float32)
        Wo = np.asarray(Wo, np.float32)
        bq16 = 16.0 * np.asarray(bq, np.float32)
        bk16 = 16.0 * np.asarray(bk, np.float32)
        bvf = np.asarray(bv, np.float32)
        tri = np.triu(np.ones((P, P), np.float32)).astype(bf)  # keep k <= q
        for c in range(NCORES):
            b = c // CPB
            hp = c % CPB
            cols = slice(hp * P, (hp + 1) * P)

            def wslice(W, dtype, scale=1.0):
                return np.ascontiguousarray(
                    (scale * W[:, cols]).reshape(DC, P, P)
                    .transpose(1, 0, 2)).astype(dtype)

            m = {
                "xb": np.ascontiguousarray(inputs[b]),
                "wq": wslice(Wq, e4, 16.0),
                "wk": wslice(Wk, e4, 16.0),
                "wv": wslice(Wv, bf),
                "wo": np.ascontiguousarray(Wo[cols, :]).astype(bf),
                "bq": np.ascontiguousarray(np.broadcast_to(
                    bq16[cols][:, None], (P, SC))),
                "bk": np.ascontiguousarray(np.broadcast_to(
                    bk16[cols][:, None], (P, SC))),
                "bv": np.ascontiguousarray(np.broadcast_to(
                    bvf[cols].reshape(1, 2, PD), (P, 2, PD))),
                "tri": np.ascontiguousarray(tri),
                "onesc": np.ones((1, PD), dtype=bf),
            }
            in_maps.append(m)
        return mode, in_maps

    # generic path (fp32/f32r), query-sharded
    if mode == "none":
        maskT = None
    elif mode == "bin":
        maskT = np.ascontiguousarray(1.0 - mask.T).astype(bf)
    else:
        maskT = np.ascontiguousarray(mask.T)
    shared = {
        "wq": _warr(Wq, np.float32), "wk": _warr(Wk, np.float32),
        "wv": _warr(Wv, np.float32), "wo": _warr(Wo, np.float32),
        "bq": _barr(bq), "bk": _barr(bk),
        "bv": np.ascontiguousarray(
            np.broadcast_to(np.asarray(bv, dtype=np.float32), (P, D))),
        "bo": np.ascontiguousarray(
            np.broadcast_to(np.asarray(bo, dtype=np.float32), (P, D))),
    }
    for c in range(NCORES):
        b = c // CPB
        rows = _q_rows(c, mode)
        m = dict(shared)
        m["ones"] = np.ones((P, H), dtype=bf)
        m["onesr"] = np.ones((1, PD), dtype=np.float32)
        m["xb"] = np.ascontiguousarray(inputs[b])
        m["xq"] = np.ascontiguousarray(inputs[b][rows])
        if maskT is not None:
            m["maskT"] = np.ascontiguousarray(maskT[:, rows])
        in_maps.append(m)
    return mode, in_maps


def assemble(results, mode):
    out = np.empty((B, S, D), dtype=np.float32)
    if mode == "tril":
        bo = _bo_stash["bo"]
        for b in range(B):
            acc = results[b * CPB]["out"].astype(np.float32).copy()
            for hp in range(1, CPB):
                acc += results[b * CPB + hp]["out"]
            out[b] = acc + bo
        return out
    for c in range(NCORES):
        b = c // CPB
        out[b, _q_rows(c, mode)] = results[c]["out"]
    return out


def kernel(inputs, mask, Wq, bq, Wk, bk, Wv, bv, Wo, bo):
    from concourse.bass_utils import run_bass_kernel_spmd

    mode, in_maps = make_in_maps(inputs, mask, Wq, bq, Wk, bk, Wv, bv, Wo, bo)
    nc = _get_prog(mode)
    res = run_bass_kernel_spmd(nc, in_maps, core_ids=list(range(NCORES)))
    return assemble(res.results, mode)
